# revision 10
# baseline (speedup 1.0000x reference)
"""Trainium2 Bass kernel for banded local attention.

Reference computation (B=2, S=2048, D=512, H=8, dh=64, local_range=7):
  q = hs @ Wq, k = hs @ Wk (per-head slices)
  scores = q k^T / sqrt(dh); w = softmax(scores) * band; w /= sum(w) + 1e-6
  ctx = w @ hs                                  -> [B, H, S, D]

Band-renormalized softmax == band-limited softmax up to the 1e-6*Z
correction (~1e-4 relative), so only the 15-diagonal band of scores is
ever computed.

Sharding (8 cores): core c -> batch b = c//4, S-half sh = (c//2)%2,
head group hg = c%2 (heads 4hg..4hg+3). Each core emits a [1024, 4, 512]
bf16 slab of unnormalized band-weighted sums plus the per-row band sums
(f32); the host divides and reassembles in f32.

Per-core tiling: 9 row tiles of P=114 (last 112). For each tile the band
j-window [i0-7, i0+107+14) spans exactly 128 rows, so the ctx contraction
is one K=128 matmul per head. Scores are computed transposed ([j, i]) so
the exp output feeds ctx directly as lhsT with no PE transposes; band
sums come from ones-vector matmuls (N=1) into spare PSUM columns.
"""

import numpy as np
import ml_dtypes

BF = ml_dtypes.bfloat16
S, D, H, DH = 2048, 512, 8, 64
NCORES = 8
SL = 1024          # rows per core (S/2)
P = 114            # row-tile height (window = P + 14 = 128)
NT = 9             # ceil(1024 / 114); last tile has 112 rows
KT = D // 128      # contraction tiles for projections
HW = 1040          # hsd width: [s0-7, s0+1033)
NEG = -10000.0

TRACE = False
LAST_RESULTS = None

_NC_CACHE = {}


def _build_nc():
    import concourse.bacc as bacc
    import concourse.mybir as mybir
    import concourse.tile as tile

    f32 = mybir.dt.float32
    bf16 = mybir.dt.bfloat16
    AF = mybir.ActivationFunctionType

    nc = bacc.Bacc("TRN2", target_bir_lowering=False, debug=False, num_devices=NCORES)

    # hs^T slice, zero-padded outside the batch: col c = hs row s0-7+c
    hsd = nc.dram_tensor("hsd", [D, HW], bf16, kind="ExternalInput").ap()
    # band windows: win[p, t, :] = hs row (s0 + 114t - 7 + p), zero-padded
    win_d = nc.dram_tensor("win", [128, NT, D], bf16, kind="ExternalInput").ap()
    # projections packed [p, kt, (q_hp0 | q_hp1 | k_hp0 | k_hp1)], q pre-scaled
    wqk = nc.dram_tensor("wqk", [128, KT, 512], bf16, kind="ExternalInput").ap()
    # consts: identity(128) | ones(1) | mask slot0 | slot1 | slot2 (114 each)
    cmask = nc.dram_tensor("cmask", [128, 129 + 3 * P], bf16, kind="ExternalInput").ap()
    out = nc.dram_tensor("out", [SL, 4, D], bf16, kind="ExternalOutput").ap()
    s_out = nc.dram_tensor("s_out", [P, NT, 4], f32, kind="ExternalOutput").ap()

    with tile.TileContext(nc) as tc:
        with (
            tc.tile_pool(name="const", bufs=1) as cpool,
            tc.tile_pool(name="ework", bufs=3) as epool,
            tc.tile_pool(name="outp", bufs=3) as opool,
            tc.tile_pool(name="pproj", bufs=1, space="PSUM") as pproj,
            tc.tile_pool(name="pscore", bufs=2, space="PSUM") as pscore,
            tc.tile_pool(name="pctx", bufs=2, space="PSUM") as pctx_pool,
        ):
            # ---- inputs, critical prefix first ----
            wqk_sb = cpool.tile([128, KT, 512], bf16)
            nc.sync.dma_start(out=wqk_sb, in_=wqk)
            hsT = cpool.tile([128, KT, HW], bf16)
            hsd_r = hsd.rearrange("(t p) s -> p t s", p=128)
            nc.sync.dma_start(out=hsT[:, :, 0:128], in_=hsd_r[:, :, 0:128])
            cm = cpool.tile([128, 129 + 3 * P], bf16)
            nc.sync.dma_start(out=cm, in_=cmask)
            id_sb = cm[:, 0:128]
            ones_sb = cm[:, 128:129]
            win = cpool.tile([128, NT, D], bf16)
            nc.sync.dma_start(out=win[:, 0:2], in_=win_d[:, 0:2])
            nc.sync.dma_start(out=hsT[:, :, 128:512], in_=hsd_r[:, :, 128:512])
            nc.sync.dma_start(out=win[:, 2:5], in_=win_d[:, 2:5])
            nc.sync.dma_start(out=hsT[:, :, 512:HW], in_=hsd_r[:, :, 512:HW])
            nc.sync.dma_start(out=win[:, 5:NT], in_=win_d[:, 5:NT])

            # qk[, 0] = qT, qk[, 1] = kT; each [128(2 heads x 64), hp, col]
            qk = cpool.tile([128, 2, 2, HW], bf16)
            s_sb = cpool.tile([P, NT, 4], f32)

            # p-state warmup: keep PE busy from wqk-arrival until the real
            # projections are ready so they dispatch at a higher clock
            dummy = pproj.tile([128, 2, 512], f32, tag="pp")
            for _ in range(2):
                nc.tensor.matmul(dummy[:, 0, :], wqk_sb[:, 0, 0:128],
                                 wqk_sb[:, 0, :], start=True, stop=True)

            evict_ctr = [0]

            def evict(o_ap, p_ap):
                """Alternate PSUM evictions between ACT and DVE."""
                use_act = evict_ctr[0] % 2 == 0
                evict_ctr[0] += 1
                if use_act:
                    nc.scalar.copy(o_ap, p_ap)
                else:
                    nc.vector.tensor_copy(o_ap, p_ap)

            def emit_proj(c0, c1, hp):
                cw = c1 - c0
                pq = pproj.tile([128, 2, 512], f32, tag="pp")
                for qi in range(2):          # 0 = q, 1 = k
                    blk = (2 * qi + hp) * 128
                    for kt in range(KT):
                        nc.tensor.matmul(
                            pq[:, qi, 0:cw], wqk_sb[:, kt, blk:blk + 128],
                            hsT[:, kt, c0:c1],
                            start=(kt == 0), stop=(kt == KT - 1),
                        )
                evict(qk[:, :, hp, c0:c1], pq[:, :, 0:cw])

            tiles_E = {}

            def emit_head(t):
                """Mask + scores (transposed [j, i]) + exp + band sums."""
                mslot = 0 if t == 0 else (1 if t < NT - 1 else 2)
                moff = 129 + mslot * P
                icol = 7 + P * t
                jcol = P * t
                psc = pscore.tile([128, 464], f32, tag="psc")
                pscv = psc[:].rearrange("p (h m) -> p h m", h=4)[:, :, 0:P]
                for h in range(4):
                    hp = h // 2
                    pr = (h % 2) * 64
                    nc.tensor.matmul(
                        pscv[:, h, :], id_sb, cm[:, moff:moff + P],
                        start=True, stop=False,
                    )
                    nc.tensor.matmul(
                        pscv[:, h, :],
                        qk[pr:pr + 64, 1, hp, jcol:jcol + 128],
                        qk[pr:pr + 64, 0, hp, icol:icol + P],
                        start=False, stop=True,
                    )
                E = epool.tile([128, 4, P], bf16, tag="E")
                nc.scalar.activation(E, pscv[:, 0:4, :], AF.Exp)
                tiles_E[t] = (E, psc)

            def emit_tail(t):
                """ctx matmuls, fused pair evictions, band sums, store."""
                E, psc = tiles_E.pop(t)
                Pt = P if t < NT - 1 else SL - P * (NT - 1)
                i0 = P * t
                o = opool.tile([P, 4, D], bf16, tag="o")
                for hp in range(2):
                    pctx = pctx_pool.tile([P, 2, D], f32, tag="pctx")
                    for hh in range(2):
                        nc.tensor.matmul(pctx[:, hh, :], E[:, 2 * hp + hh, :],
                                         win[:, t, :], start=True, stop=True)
                    evict(o[:, 2 * hp:2 * hp + 2, :], pctx)
                # band sums into the spare psum columns, then stage to SBUF
                for h in range(4):
                    nc.tensor.matmul(psc[0:P, 456 + h:457 + h], E[:, h, :],
                                     ones_sb, start=True, stop=True)
                nc.vector.tensor_copy(s_sb[:, t, :], psc[0:P, 456:460])
                nc.sync.dma_start(out=out[i0:i0 + Pt], in_=o[0:Pt])

            # ---- software-pipelined emission ----
            emit_proj(0, 128, 0)
            emit_proj(0, 128, 1)
            emit_head(0)
            emit_proj(128, 512, 0)
            emit_proj(128, 512, 1)
            emit_head(1)
            emit_tail(0)
            emit_proj(512, 1024, 0)
            emit_head(2)
            emit_tail(1)
            emit_proj(512, 1024, 1)
            emit_head(3)
            emit_tail(2)
            nc.gpsimd.dma_start(out=s_out[:, 0:3], in_=s_sb[:, 0:3])
            emit_proj(1024, HW, 0)
            emit_proj(1024, HW, 1)
            for t in range(4, NT):
                emit_head(t)
                emit_tail(t - 1)
                if t == 6:
                    nc.gpsimd.dma_start(out=s_out[:, 3:6], in_=s_sb[:, 3:6])
            emit_tail(NT - 1)
            nc.gpsimd.dma_start(out=s_out[:, 6:NT], in_=s_sb[:, 6:NT])

    nc.compile()
    return nc


def _get_nc():
    if "nc" not in _NC_CACHE:
        _NC_CACHE["nc"] = _build_nc()
    return _NC_CACHE["nc"]


def _band_mask(jmin, jmax):
    """[128, P] bf16 mask in [j, i] orientation: 0 inside band, NEG outside."""
    j = np.arange(128)[:, None]
    i = np.arange(P)[None, :]
    valid = (j - i >= 0) & (j - i <= 14) & (j >= jmin) & (j <= jmax)
    return np.where(valid, 0.0, NEG).astype(BF)


def kernel(hidden_states, Wq, Wk):
    global LAST_RESULTS
    from concourse import bass_utils

    B = hidden_states.shape[0]
    hs_bf = np.asarray(hidden_states).astype(BF)
    wq = np.asarray(Wq).astype(np.float32) * (1.0 / (DH ** 0.5))
    wk = np.asarray(Wk).astype(np.float32)

    in_maps = []
    for c in range(NCORES):
        b = c // 4
        sh = (c // 2) % 2
        hg = c % 2
        s0 = sh * SL

        pad = np.zeros((S + 16, D), BF)
        pad[7:7 + S] = hs_bf[b]

        hsd = np.ascontiguousarray(pad[s0:s0 + HW].T)            # [512, 1040]
        win = np.ascontiguousarray(
            np.stack([pad[s0 + P * t: s0 + P * t + 128] for t in range(NT)], axis=1)
        )                                                         # [128, 9, 512]

        wq_s = wq[:, hg * 256:(hg + 1) * 256]
        wk_s = wk[:, hg * 256:(hg + 1) * 256]
        packed = np.concatenate(
            [wq_s[:, 0:128], wq_s[:, 128:256], wk_s[:, 0:128], wk_s[:, 128:256]],
            axis=1,
        ).astype(BF)                                              # [512, 512]
        wqk = np.ascontiguousarray(packed.reshape(KT, 128, 512).transpose(1, 0, 2))

        # masks: slot0 (t=0), slot1 (interior), slot2 (t=8); j bounds clamp
        # the window to the batch (zero-padded rows must not survive exp)
        m0 = _band_mask(7 if sh == 0 else -1, 999)
        m1 = _band_mask(-1, 999)
        m2 = _band_mask(-1, 118 if sh == 1 else 999)
        cmask = np.concatenate(
            [np.eye(128, dtype=BF), np.ones((128, 1), BF), m0, m1, m2], axis=1
        )

        in_maps.append({"hsd": hsd, "win": win, "wqk": wqk, "cmask": cmask})

    nc = _get_nc()
    res = bass_utils.run_bass_kernel_spmd(
        nc, in_maps, core_ids=list(range(NCORES)), trace=TRACE,
    )
    LAST_RESULTS = res

    out = np.empty((B, H, S, D), np.float32)
    for c in range(NCORES):
        b = c // 4
        sh = (c // 2) % 2
        hg = c % 2
        s0 = sh * SL
        slab = np.asarray(res.results[c]["out"]).astype(np.float32)  # [1024, 4, 512]
        s = np.asarray(res.results[c]["s_out"]).astype(np.float32)   # [114, 9, 4]
        s = s.transpose(1, 0, 2).reshape(NT * P, 4)[:SL]             # [1024, 4]
        slab /= (s + 1e-6)[:, :, None]
        out[b, 4 * hg:4 * hg + 4, s0:s0 + SL] = slab.transpose(1, 0, 2)
    return out


# revision 16
# speedup vs baseline: 1.0184x; 1.0184x over previous
"""Trainium2 Bass kernel for banded local attention.

Reference computation (B=2, S=2048, D=512, H=8, dh=64, local_range=7):
  q = hs @ Wq, k = hs @ Wk (per-head slices)
  scores = q k^T / sqrt(dh); w = softmax(scores) * band; w /= sum(w) + 1e-6
  ctx = w @ hs                                  -> [B, H, S, D]

Band-renormalized softmax == band-limited softmax up to the 1e-6*Z
correction (~1e-4 relative), so only the 15-diagonal band of scores is
ever computed.

Sharding (8 cores): core c -> batch b = c//4, S-half sh = (c//2)%2,
head group hg = c%2 (heads 4hg..4hg+3). Each core emits a [1024, 4, 512]
bf16 slab of unnormalized band-weighted sums plus the per-row band sums
(f32); the host divides and reassembles in f32.

Per-core tiling: 9 row tiles of P=114 (last 112). For each tile the band
j-window [i0-7, i0+107+14) spans exactly 128 rows, so the ctx contraction
is one K=128 matmul per head. Scores are computed transposed ([j, i]) so
the exp output feeds ctx directly as lhsT with no PE transposes; the band
mask is applied post-exp as a 0/1 multiply on GpSimd; band sums come from
ones-vector matmuls (N=1) into spare PSUM columns.
"""

import numpy as np
import ml_dtypes

BF = ml_dtypes.bfloat16
S, D, H, DH = 2048, 512, 8, 64
NCORES = 8
SL = 1024          # rows per core (S/2)
P = 114            # row-tile height (window = P + 14 = 128)
NT = 9             # ceil(1024 / 114); last tile has 112 rows
KT = D // 128      # contraction tiles for projections
HW = 1040          # hsd width: [s0-7, s0+1033)

TRACE = False
LAST_RESULTS = None

_NC_CACHE = {}


def _build_nc():
    import concourse.bacc as bacc
    import concourse.mybir as mybir
    import concourse.tile as tile

    f32 = mybir.dt.float32
    bf16 = mybir.dt.bfloat16
    AF = mybir.ActivationFunctionType

    nc = bacc.Bacc("TRN2", target_bir_lowering=False, debug=False, num_devices=NCORES)

    # hs^T slice, zero-padded outside the batch: col c = hs row s0-7+c
    hsd = nc.dram_tensor("hsd", [D, HW], bf16, kind="ExternalInput").ap()
    # band windows: win[p, t, :] = hs row (s0 + 114t - 7 + p), zero-padded
    win_d = nc.dram_tensor("win", [128, NT, D], bf16, kind="ExternalInput").ap()
    # projections packed [p, kt, (q_hp0 | q_hp1 | k_hp0 | k_hp1)], q pre-scaled
    wqk = nc.dram_tensor("wqk", [128, KT, 512], bf16, kind="ExternalInput").ap()
    # consts: identity(128) | 0/1+NEG band masks slot0..2 (114 each) | ones(2)
    cmask = nc.dram_tensor("cmask", [128, 128 + 3 * P + 2], bf16, kind="ExternalInput").ap()
    out = nc.dram_tensor("out", [SL, 4, D], bf16, kind="ExternalOutput").ap()
    s_out = nc.dram_tensor("s_out", [P, NT, 4], f32, kind="ExternalOutput").ap()

    with tile.TileContext(nc) as tc:
        with (
            tc.tile_pool(name="const", bufs=1) as cpool,
            tc.tile_pool(name="ework", bufs=3) as epool,
            tc.tile_pool(name="outp", bufs=3) as opool,
            tc.tile_pool(name="pproj", bufs=1, space="PSUM") as pproj,
            tc.tile_pool(name="pscore", bufs=2, space="PSUM") as pscore,
            tc.tile_pool(name="pctx", bufs=2, space="PSUM") as pctx_pool,
        ):
            # ---- inputs, critical prefix first ----
            wqk_sb = cpool.tile([128, KT, 512], bf16)
            nc.sync.dma_start(out=wqk_sb, in_=wqk)
            hsT = cpool.tile([128, KT, HW], bf16)
            hsd_r = hsd.rearrange("(t p) s -> p t s", p=128)
            nc.sync.dma_start(out=hsT[:, :, 0:256], in_=hsd_r[:, :, 0:256])
            cm = cpool.tile([128, 128 + 3 * P + 2], bf16)
            nc.sync.dma_start(out=cm, in_=cmask)
            id_sb = cm[:, 0:128]
            ones_sb = cm[:, 128 + 3 * P:128 + 3 * P + 1]
            win = cpool.tile([128, NT, D], bf16)
            nc.sync.dma_start(out=win[:, 0:2], in_=win_d[:, 0:2])
            nc.sync.dma_start(out=hsT[:, :, 256:512], in_=hsd_r[:, :, 256:512])
            nc.sync.dma_start(out=win[:, 2:5], in_=win_d[:, 2:5])
            nc.sync.dma_start(out=hsT[:, :, 512:HW], in_=hsd_r[:, :, 512:HW])
            nc.sync.dma_start(out=win[:, 5:NT], in_=win_d[:, 5:NT])

            # qk[, 0] = qT, qk[, 1] = kT; each [128(2 heads x 64), hp, col]
            qk = cpool.tile([128, 2, 2, HW], bf16)
            s_sb = cpool.tile([P, NT, 4], f32)

            # p-state warmup: keep PE busy from wqk-arrival until the real
            # projections are ready so they dispatch at a higher clock
            dummy = pproj.tile([128, 2, 512], f32, tag="pp")
            for _ in range(2):
                nc.tensor.matmul(dummy[:, 0, :], wqk_sb[:, 0, 0:128],
                                 wqk_sb[:, 0, :], start=True, stop=True)

            def emit_proj(c0, c1, hp, evict_eng):
                cw = c1 - c0
                pq = pproj.tile([128, 2, 512], f32, tag="pp")
                for qi in range(2):          # 0 = q, 1 = k
                    blk = (2 * qi + hp) * 128
                    for kt in range(KT):
                        nc.tensor.matmul(
                            pq[:, qi, 0:cw], wqk_sb[:, kt, blk:blk + 128],
                            hsT[:, kt, c0:c1],
                            start=(kt == 0), stop=(kt == KT - 1),
                        )
                if evict_eng == "act":
                    nc.scalar.copy(qk[:, :, hp, c0:c1], pq[:, :, 0:cw])
                else:
                    nc.vector.tensor_copy(qk[:, :, hp, c0:c1], pq[:, :, 0:cw])

            tiles_E = {}

            def emit_head(t):
                """Scores (transposed [j, i]) + exp + GpSimd band masking."""
                mslot = 0 if t == 0 else (1 if t < NT - 1 else 2)
                moff = 128 + mslot * P
                icol = 7 + P * t
                jcol = P * t
                psc = pscore.tile([128, 464], f32, tag="psc")
                pscv = psc[:].rearrange("p (h m) -> p h m", h=4)[:, :, 0:P]
                for h in range(4):
                    hp = h // 2
                    pr = (h % 2) * 64
                    nc.tensor.matmul(
                        pscv[:, h, :], id_sb, cm[:, moff:moff + P],
                        start=True, stop=False,
                    )
                    nc.tensor.matmul(
                        pscv[:, h, :],
                        qk[pr:pr + 64, 1, hp, jcol:jcol + 128],
                        qk[pr:pr + 64, 0, hp, icol:icol + P],
                        start=False, stop=True,
                    )
                Em = epool.tile([128, 4, P], bf16, tag="Em")
                nc.scalar.activation(Em, pscv[:, 0:4, :], AF.Exp)
                tiles_E[t] = (Em, psc)

            def emit_tail(t):
                """ctx matmuls, band sums, fused pair evictions, store."""
                Em, psc = tiles_E.pop(t)
                Pt = P if t < NT - 1 else SL - P * (NT - 1)
                i0 = P * t
                o = opool.tile([P, 4, D], bf16, tag="o")
                pcs = []
                for hp in range(2):
                    pctx = pctx_pool.tile([P, 2, D], f32, tag="pctx")
                    for hh in range(2):
                        nc.tensor.matmul(pctx[:, hh, :], Em[:, 2 * hp + hh, :],
                                         win[:, t, :], start=True, stop=True)
                    pcs.append(pctx)
                # band sums into the spare psum columns, then stage to SBUF
                for h in range(4):
                    nc.tensor.matmul(psc[0:P, 456 + h:457 + h], Em[:, h, :],
                                     ones_sb, start=True, stop=True)
                nc.vector.tensor_copy(s_sb[:, t, :], psc[0:P, 456:460])
                nc.scalar.copy(o[:, 0:2, :], pcs[0])
                nc.vector.tensor_copy(o[:, 2:4, :], pcs[1])
                nc.sync.dma_start(out=out[i0:i0 + Pt], in_=o[0:Pt])

            # ---- software-pipelined emission ----
            emit_proj(0, 256, 0, "act")
            emit_proj(0, 256, 1, "dve")
            emit_head(0)
            emit_proj(256, 512, 0, "dve")
            emit_proj(256, 512, 1, "dve")
            emit_head(1)
            emit_tail(0)
            emit_proj(512, 1024, 0, "dve")
            emit_head(2)
            emit_tail(1)
            emit_proj(512, 1024, 1, "dve")
            emit_head(3)
            emit_tail(2)
            nc.sync.dma_start(out=s_out[:, 0:3], in_=s_sb[:, 0:3])
            emit_proj(1024, HW, 0, "act")
            emit_proj(1024, HW, 1, "dve")
            for t in range(4, NT):
                emit_head(t)
                emit_tail(t - 1)
                if t == 6:
                    nc.sync.dma_start(out=s_out[:, 3:6], in_=s_sb[:, 3:6])
            emit_tail(NT - 1)
            nc.sync.dma_start(out=s_out[:, 6:NT], in_=s_sb[:, 6:NT])

    nc.compile()
    return nc


def _get_nc():
    if "nc" not in _NC_CACHE:
        _NC_CACHE["nc"] = _build_nc()
    return _NC_CACHE["nc"]


def _band_mask(jmin, jmax):
    """[128, P] bf16 0/1 mask in [j, i] orientation."""
    j = np.arange(128)[:, None]
    i = np.arange(P)[None, :]
    valid = (j - i >= 0) & (j - i <= 14) & (j >= jmin) & (j <= jmax)
    return valid.astype(BF)


def kernel(hidden_states, Wq, Wk):
    global LAST_RESULTS
    from concourse import bass_utils

    B = hidden_states.shape[0]
    hs_bf = np.asarray(hidden_states).astype(BF)
    wq = np.asarray(Wq).astype(np.float32) * (1.0 / (DH ** 0.5))
    wk = np.asarray(Wk).astype(np.float32)

    in_maps = []
    for c in range(NCORES):
        b = c // 4
        sh = (c // 2) % 2
        hg = c % 2
        s0 = sh * SL

        pad = np.zeros((S + 16, D), BF)
        pad[7:7 + S] = hs_bf[b]

        hsd = np.ascontiguousarray(pad[s0:s0 + HW].T)            # [512, 1040]
        win = np.ascontiguousarray(
            np.stack([pad[s0 + P * t: s0 + P * t + 128] for t in range(NT)], axis=1)
        )                                                         # [128, 9, 512]

        wq_s = wq[:, hg * 256:(hg + 1) * 256]
        wk_s = wk[:, hg * 256:(hg + 1) * 256]
        packed = np.concatenate(
            [wq_s[:, 0:128], wq_s[:, 128:256], wk_s[:, 0:128], wk_s[:, 128:256]],
            axis=1,
        ).astype(BF)                                              # [512, 512]
        wqk = np.ascontiguousarray(packed.reshape(KT, 128, 512).transpose(1, 0, 2))

        # masks: slot0 (t=0), slot1 (interior), slot2 (t=8); j bounds clamp
        # the window to the batch (zero-padded rows must not survive)
        m0 = _band_mask(7 if sh == 0 else -1, 999)
        m1 = _band_mask(-1, 999)
        m2 = _band_mask(-1, 118 if sh == 1 else 999)
        neg = np.float32(-10000.0)
        m0 = np.where(m0 > 0, 0.0, neg).astype(BF)
        m1 = np.where(m1 > 0, 0.0, neg).astype(BF)
        m2 = np.where(m2 > 0, 0.0, neg).astype(BF)
        cmask = np.concatenate([np.eye(128, dtype=BF), m0, m1, m2,
                                np.ones((128, 2), BF)], axis=1)

        in_maps.append({"hsd": hsd, "win": win, "wqk": wqk, "cmask": cmask})

    nc = _get_nc()
    res = bass_utils.run_bass_kernel_spmd(
        nc, in_maps, core_ids=list(range(NCORES)), trace=TRACE,
    )
    LAST_RESULTS = res

    out = np.empty((B, H, S, D), np.float32)
    for c in range(NCORES):
        b = c // 4
        sh = (c // 2) % 2
        hg = c % 2
        s0 = sh * SL
        slab = np.asarray(res.results[c]["out"]).astype(np.float32)  # [1024, 4, 512]
        s = np.asarray(res.results[c]["s_out"]).astype(np.float32)   # [114, 9, 4]
        s = s.transpose(1, 0, 2).reshape(NT * P, 4)[:SL]             # [1024, 4]
        slab /= (s + 1e-6)[:, :, None]
        out[b, 4 * hg:4 * hg + 4, s0:s0 + SL] = slab.transpose(1, 0, 2)
    return out
